# revision 5
# baseline (speedup 1.0000x reference)
"""Causal self-attention Bass/Tile kernel for TRN2, data-parallel over 8 NeuronCores.

Shapes (hardcoded): x [16, 1024, 1024] f32, W_attn [1024, 3072], b_attn [3072],
W_proj [1024, 1024], b_proj [1024].  16 heads, head dim 64.
Each core processes 2 batch elements end-to-end; no collectives.

Per-core pipeline (per batch):
  1. x -> x^T via PE transposes, 8 per PSUM bank, one batched DVE evict per
     token tile.
  2. v = (x^T tile).T @ W_v (natural form) into vext (bf16) with a ones
     column appended per head for softmax denominators; W_proj preloaded.
  3. Per head pair hp: q^T,k^T = (W tile).T @ x^T with one LDWEIGHTS per
     (mt,k) serving both 512-token chunks; scores^T = k^T.T @ q^T with the
     two heads row-packed (tile_position) into one 2-bank PSUM tile so a
     single EXP activation (scale=1/8 folded) covers both; pt stored
     column-shifted so every diagonal 128x128 block lands at cols 0:128,
     letting one strided DVE mul apply the causal mask for all 4 diagonal
     tiles x 2 heads at once; AV = vext.T @ P^T accumulated in PSUM with
     row 64 collecting denominators; reciprocal_approx_fast on that row,
     gpsimd partition_broadcast, and a fused multiply-evict into yT.
  4. out = (y^T tile).T @ W_proj + b_proj, 8-chunk PSUM accumulation,
     streamed to HBM.
"""
import sys

sys.path.insert(0, "/opt/trn_rl_repo")

from contextlib import ExitStack

import numpy as np

import concourse.bass as bass
import concourse.mybir as mybir
import concourse.tile as tile
from concourse import bacc
from concourse.bass_utils import run_bass_kernel_spmd
from concourse.masks import make_identity, make_upper_triangular

import concourse.hw_specs as hw_specs

F32 = mybir.dt.float32
BF16 = mybir.dt.bfloat16
EXP = mybir.ActivationFunctionType.Exp
LOG = mybir.ActivationFunctionType.Ln

# Force Exp and Ln to resolve to the combined natural_log_exp_and_others
# activation-table set so softmax exp and the ln/exp reciprocal share one
# table load instead of thrashing 2.7us ACT_TABLE_LOADs per switch.
_orig_gat = hw_specs.get_activation_tables
def _patched_gat(arch):
    t = _orig_gat(arch)
    if "natural_log_exp_and_others" in t:
        t["exp_and_others"].discard(EXP)
        t["natural_log"].discard(LOG)
    return t
hw_specs.get_activation_tables = _patched_gat
bacc.get_activation_tables = _patched_gat

N_CORES = 8
B, T, C = 16, 1024, 1024
H, DH = 16, 64
BL = B // N_CORES          # batches per core
TT = T // 128              # token tiles (8)
KO = C // 128              # contraction chunks (8)
NQ = T // 512              # 512-wide token chunks (2)
SCALE = 1.0 / 8.0          # 1/sqrt(64)


def _emit(nc, tc, x_d, wattn_d, battn_d, wproj_d, bproj_d, out_d):
    with ExitStack() as ctx:
        const = ctx.enter_context(tc.tile_pool(name="const", bufs=1))
        xT_pool = ctx.enter_context(tc.tile_pool(name="xT", bufs=2))
        yT_pool = ctx.enter_context(tc.tile_pool(name="yT", bufs=2))
        vext_pool = ctx.enter_context(tc.tile_pool(name="vext", bufs=2))
        qk_pool = ctx.enter_context(tc.tile_pool(name="qk", bufs=3))
        pt_pool = ctx.enter_context(tc.tile_pool(name="pt", bufs=2))
        w_pool = ctx.enter_context(tc.tile_pool(name="w", bufs=8))
        norm_pool = ctx.enter_context(tc.tile_pool(name="norm", bufs=1))
        dram_pool = ctx.enter_context(tc.tile_pool(name="dram", bufs=4, space="DRAM"))
        wp_pool = ctx.enter_context(tc.tile_pool(name="wp", bufs=16))
        # PSUM: psS 2x[128,2,512]f32 (4 banks) + psAV 2x[128,512]f32 (2) +
        # psM 2x2KB (2) = 8 banks
        psS = ctx.enter_context(tc.tile_pool(name="psS", bufs=2, space="PSUM"))
        psAV = ctx.enter_context(tc.tile_pool(name="psAV", bufs=2, space="PSUM"))
        psM = ctx.enter_context(tc.tile_pool(name="psM", bufs=2, space="PSUM"))

        # ---- constants ----
        ident = const.tile([128, 128], BF16)
        make_identity(nc, ident)
        # upper-tri (incl diag) ones in (k, q) sense for diagonal score blocks
        zmask = const.tile([128, 128], BF16)
        make_upper_triangular(nc, zmask, val=1.0, diag=True)
        ones_c = const.tile([128, 1], F32)
        nc.vector.memset(ones_c, 1.0)
        # biases
        b_qk = const.tile([128, 16], F32)
        nc.sync.dma_start(b_qk, battn_d[0 : 2 * C].rearrange("(m p) -> p m", p=128))
        bv_b = const.tile([128, C], F32)
        nc.sync.dma_start(bv_b, battn_d[None, 2 * C : 3 * C].to_broadcast((128, C)))
        bpj_b = const.tile([128, C], F32)
        nc.sync.dma_start(bpj_b, bproj_d[None, :].to_broadcast((128, C)))

        def ph1(b):
            # ---- phase 1: x^T; 8 transposes per PSUM bank, 1 evict per tt ----
            xT = xT_pool.tile([128, KO, T], BF16, tag="xT", name=f"xT{b}")
            for tt in range(TT):
                xin = w_pool.tile([128, C], BF16, tag="w", name=f"xin{b}_{tt}")
                nc.sync.dma_start(xin, x_d[b, tt * 128 : (tt + 1) * 128, :])
                tp = psM.tile([128, 1024], BF16, tag="m", name=f"tp{b}_{tt}")
                for co in range(KO):
                    nc.tensor.transpose(
                        tp[:, co * 128 : (co + 1) * 128],
                        xin[:, co * 128 : (co + 1) * 128],
                        ident,
                    )
                nc.vector.tensor_copy(
                    xT[:, :, tt * 128 : (tt + 1) * 128],
                    tp.rearrange("p (co c) -> p co c", c=128),
                )
            return xT

        def ph2(b, xT):
            # ---- phase 2: v (natural layout) into vext with ones column ----
            vext = vext_pool.tile([128, TT, H, DH + 1], BF16, tag="vext", name=f"vext{b}")
            for nn in range(NQ):
                wv = []
                for k in range(KO):
                    wt = w_pool.tile([128, 512], BF16, tag="w", name=f"wv{b}_{nn}_{k}")
                    nc.sync.dma_start(
                        wt,
                        wattn_d[
                            k * 128 : (k + 1) * 128,
                            2 * C + nn * 512 : 2 * C + (nn + 1) * 512,
                        ],
                    )
                    wv.append(wt)
                for m in range(TT):
                    ps = psM.tile([128, 512], F32, tag="m", name=f"vps{b}_{nn}_{m}")
                    for k in range(KO):
                        nc.tensor.matmul(
                            ps,
                            xT[:, k, m * 128 : (m + 1) * 128],
                            wv[k],
                            start=(k == 0),
                            stop=(k == KO - 1),
                        )
                    nc.vector.tensor_add(
                        vext[:, m, nn * 8 : (nn + 1) * 8, 0:DH],
                        ps.rearrange("p (h d) -> p h d", d=DH),
                        bv_b[:, nn * 512 : (nn + 1) * 512].rearrange(
                            "p (h d) -> p h d", d=DH
                        ),
                    )
            nc.vector.tensor_copy(
                vext[:, :, :, DH : DH + 1],
                ones_c[:, 0:1, None].to_broadcast((128, TT, H, 1)),
            )
            # preload W_proj so phase-4 DMAs aren't queued behind attention DMAs
            wp_all = []
            for nn in range(NQ):
                wpn = []
                for k in range(KO):
                    wt = wp_pool.tile([128, 512], BF16, tag="wp", name=f"wp{b}_{nn}_{k}")
                    nc.sync.dma_start(
                        wt, wproj_d[k * 128 : (k + 1) * 128, nn * 512 : (nn + 1) * 512]
                    )
                    wpn.append(wt)
                wp_all.append(wpn)
            return vext, wp_all

        def ph3(b, xT, vext):
            # ---- phase 3: per head pair: q^T/k^T, scores, softmax, AV ----
            yT_lo = yT_pool.tile([128, KO // 2, T], BF16, tag="yTlo", name=f"yTlo{b}")
            yT_hi = yT_pool.tile([128, KO // 2, T], BF16, tag="yThi", name=f"yThi{b}")
            for hp in range(KO):
                yT = yT_lo if hp < KO // 2 else yT_hi
                hpo = hp % (KO // 2)
                # q^T/k^T GEMM: one LDWEIGHTS per (mt, k) serves both nn chunks
                qk = qk_pool.tile([128, 2, T], BF16, tag="qk", name=f"qk{b}_{hp}")
                for which, mt in ((0, hp), (1, 8 + hp)):
                    wt = w_pool.tile([128, KO, 128], BF16, tag="w", name=f"wqk{b}_{mt}")
                    nc.sync.dma_start(
                        wt,
                        wattn_d[:, mt * 128 : (mt + 1) * 128].rearrange(
                            "(ko p) m -> p ko m", p=128
                        ),
                    )
                    pss = [
                        psM.tile([128, 512], F32, tag="m", name=f"qkps{b}_{mt}_{nn}")
                        for nn in range(NQ)
                    ]
                    for k in range(KO):
                        for nn in range(NQ):
                            nc.tensor.matmul(
                                pss[nn],
                                wt[:, k, :],
                                xT[:, k, nn * 512 : (nn + 1) * 512],
                                start=(k == 0),
                                stop=(k == KO - 1),
                            )
                    for nn in range(NQ):
                        nc.vector.tensor_add(
                            qk[:, which, nn * 512 : (nn + 1) * 512],
                            pss[nn],
                            b_qk[:, mt : mt + 1].to_broadcast((128, 512)),
                        )

                for qc in range(NQ):
                    KT = 4 * qc + 4
                    # pt column-shifted: data for kt stored at cols [0, 512-st)
                    pt = pt_pool.tile(
                        [128, 2, KT, 512], BF16, tag=f"pt{qc}", name=f"pt{b}_{hp}_{qc}"
                    )
                    for kt in range(KT):
                        j = kt - 4 * qc
                        st = 0 if j < 0 else j * 128
                        ss = psS.tile([128, 2, 512], F32, tag="s", name=f"sc{b}_{hp}_{qc}_{kt}")
                        for h2 in range(2):
                            nc.tensor.matmul(
                                ss[:, h2, st:512],
                                qk[64 * h2 : 64 * h2 + 64, 1, kt * 128 : (kt + 1) * 128],
                                qk[
                                    64 * h2 : 64 * h2 + 64,
                                    0,
                                    qc * 512 + st : (qc + 1) * 512,
                                ],
                                start=True,
                                stop=True,
                                tile_position=(64 * h2, 0),
                            )
                        nc.scalar.activation(
                            pt[:, :, kt, 0 : 512 - st], ss[:, :, st:512], EXP, scale=SCALE
                        )
                    # causal mask: all diagonal blocks sit at cols 0:128 of
                    # their (shifted) pt rows -> one strided mul for 4 kt x 2 h2
                    dlo = 4 * qc
                    nc.vector.tensor_mul(
                        pt[:, :, dlo : dlo + 4, 0:128],
                        pt[:, :, dlo : dlo + 4, 0:128],
                        zmask[:, None, None, :].to_broadcast((128, 2, 4, 128)),
                    )
                    ypss = []
                    for h2 in range(2):
                        h = 2 * hp + h2
                        yps = psAV.tile([128, 512], F32, tag="av", name=f"av{b}_{hp}_{qc}_{h2}")
                        for kt in range(KT):
                            j = kt - 4 * qc
                            st = 0 if j < 0 else j * 128
                            nc.tensor.matmul(
                                yps[0 : DH + 1, st:512],
                                vext[:, kt, h, :],
                                pt[:, h2, kt, 0 : 512 - st],
                                start=(kt == 0),
                                stop=(kt == KT - 1),
                            )
                        ypss.append(yps)
                    # softmax denominators: gather the two ones-rows, recip
                    # via ln/exp on ScalarE (same table set as softmax exp),
                    # DRAM-bounce broadcast, fused scale-evict into yT
                    sg = norm_pool.tile([128, 512], F32, tag="sg", name=f"sg{b}_{hp}_{qc}")
                    nc.vector.tensor_copy(sg[0:1, :], ypss[0][DH : DH + 1, :])
                    nc.vector.tensor_copy(sg[32:33, :], ypss[1][DH : DH + 1, :])
                    lnb = norm_pool.tile([128, 512], F32, tag="lnb", name=f"ln{b}_{hp}_{qc}")
                    nc.scalar.activation(lnb, sg, LOG)
                    nc.scalar.activation(sg, lnb, EXP, scale=-1.0)
                    rd = dram_pool.tile([2, 512], F32, tag="rd", name=f"rd{b}_{hp}_{qc}")
                    nc.sync.dma_start(rd, sg.rearrange("(g r) c -> g r c", g=4)[0:2, 0, :])
                    rb = norm_pool.tile([128, 2, 512], F32, tag="rb", name=f"rb{b}_{hp}_{qc}")
                    nc.sync.dma_start(rb, rd[None, :, :].to_broadcast((128, 2, 512)))
                    for h2 in range(2):
                        nc.vector.tensor_mul(
                            yT[64 * h2 : 64 * h2 + 64, hpo, qc * 512 : (qc + 1) * 512],
                            ypss[h2][0:DH, :],
                            rb[0:64, h2, :],
                        )
            return yT_lo, yT_hi

        def ph4(b, yT_lo, yT_hi, wp_all):
            # ---- phase 4: out = y @ W_proj + b_proj; 8-chunk PSUM chains ----
            for nn in range(NQ):
                wp = wp_all[nn]
                for m in range(TT):
                    ps = psM.tile([128, 512], F32, tag="m", name=f"pj{b}_{nn}_{m}")
                    for k in range(KO):
                        yT = yT_lo if k < KO // 2 else yT_hi
                        nc.tensor.matmul(
                            ps,
                            yT[:, k % (KO // 2), m * 128 : (m + 1) * 128],
                            wp[k],
                            start=(k == 0),
                            stop=(k == KO - 1),
                        )
                    osb = w_pool.tile([128, 512], F32, tag="w", name=f"os{b}_{nn}_{m}")
                    nc.vector.tensor_add(osb, ps, bpj_b[:, nn * 512 : (nn + 1) * 512])
                    nc.sync.dma_start(
                        out_d[b, m * 128 : (m + 1) * 128, nn * 512 : (nn + 1) * 512],
                        osb,
                    )

        # software pipeline across the two batches
        xT0 = ph1(0)
        vext0, wp0 = ph2(0, xT0)
        y0lo, y0hi = ph3(0, xT0, vext0)
        xT1 = ph1(1)
        ph4(0, y0lo, y0hi, wp0)
        vext1, wp1 = ph2(1, xT1)
        y1lo, y1hi = ph3(1, xT1, vext1)
        ph4(1, y1lo, y1hi, wp1)


_CACHE = {}


def _build():
    if "nc" in _CACHE:
        return _CACHE["nc"]
    nc = bacc.Bacc("TRN2", target_bir_lowering=False, debug=False)
    x_d = nc.dram_tensor("x", [BL, T, C], BF16, kind="ExternalInput").ap()
    wattn_d = nc.dram_tensor("W_attn", [C, 3 * C], BF16, kind="ExternalInput").ap()
    battn_d = nc.dram_tensor("b_attn", [3 * C], F32, kind="ExternalInput").ap()
    wproj_d = nc.dram_tensor("W_proj", [C, C], BF16, kind="ExternalInput").ap()
    bproj_d = nc.dram_tensor("b_proj", [C], F32, kind="ExternalInput").ap()
    out_d = nc.dram_tensor("out", [BL, T, C], F32, kind="ExternalOutput").ap()
    with tile.TileContext(nc) as tc:
        _emit(nc, tc, x_d, wattn_d, battn_d, wproj_d, bproj_d, out_d)
    nc.compile()
    _CACHE["nc"] = nc
    return nc


def kernel(x, W_attn, b_attn, W_proj, b_proj, _trace=False):
    nc = _build()
    import ml_dtypes

    x = np.ascontiguousarray(np.asarray(x, dtype=np.float32).astype(ml_dtypes.bfloat16))
    W_attn = np.ascontiguousarray(np.asarray(W_attn, dtype=np.float32).astype(ml_dtypes.bfloat16))
    b_attn = np.ascontiguousarray(np.asarray(b_attn, dtype=np.float32))
    W_proj = np.ascontiguousarray(np.asarray(W_proj, dtype=np.float32).astype(ml_dtypes.bfloat16))
    b_proj = np.ascontiguousarray(np.asarray(b_proj, dtype=np.float32))
    in_maps = [
        {
            "x": x[i * BL : (i + 1) * BL],
            "W_attn": W_attn,
            "b_attn": b_attn,
            "W_proj": W_proj,
            "b_proj": b_proj,
        }
        for i in range(N_CORES)
    ]
    res = run_bass_kernel_spmd(nc, in_maps, core_ids=list(range(N_CORES)), trace=_trace)
    out = np.concatenate([res.results[i]["out"] for i in range(N_CORES)], axis=0)
    if _trace:
        kernel.last_results = res
    return out
